# revision 27
# baseline (speedup 1.0000x reference)
"""AlphaNet forward pass on 8 Trainium2 NeuronCores (data-parallel over batch).

Pipeline per core (512 samples):
  DVE: rolling-window stats (corr/cov/std/zscore/return/decaylinear) in an
       unscaled "mine" form, written into a padded [272 rows x 16 win] buffer.
  PE : transpose features to [rows, samples]; conv(1x3)+BN folded into small
       sparse matmuls (A pieces) feeding fc1's K-tiles; fc1/fc2/fc3 in bf16.
  ACT: relu/sigmoid + bias epilogues (PSUM f32 -> SBUF bf16).
All per-row constant factors (BatchNorm affine, 1/9, 0.9, 0.3, ret's -1, ...)
are folded into the host-built conv matrix A and per-row bias.
fc1 weight is host-permuted/transposed/bf16-cast and streamed once per core.
"""
import sys
for _p in ("/opt/trn_rl_repo", "/root/.axon_site/_ro/trn_rl_repo"):
    if _p not in sys.path:
        sys.path.append(_p)

from contextlib import ExitStack

import numpy as np
import ml_dtypes

import concourse.bass as bass
import concourse.tile as tile
from concourse import bacc, mybir
from concourse.bass_utils import run_bass_kernel_spmd
from concourse.masks import make_identity

bf16 = ml_dtypes.bfloat16
dt = mybir.dt

# ---- problem constants (hardcoded; must match the AlphaNet reference) ----
NFULL = 4096
NCORES = 8
NSH = NFULL // NCORES        # 512 samples per core
F, W, S = 15, 120, 10
NW = W // S                  # 12
HP = 270                     # stat rows
NROW_PAD, WPAD = 272, 16
GROWS = NROW_PAD * WPAD      # 4352 = 34*128
NGT = GROWS // 128           # 34 transposed-feature tiles
K1 = 43200
K1PAD = 43264                # 338*128
NT = K1PAD // 128            # 338
BN_EPS = 1e-5
NB = NSH // 128              # 4 sample blocks per core


# ------------------------- host-side preparation -------------------------

def _mine_row_tables():
    cb, pairs, base = {}, [], 0
    for d in range(1, 15):
        cb[d] = base
        for i in range(0, 15 - d):
            pairs.append((i, i + d))
        base += 15 - d
    return pairs, cb


def _ref_perm():
    pairs, _ = _mine_row_tables()
    II, JJ = np.triu_indices(F, k=1)
    p2r = {(int(i), int(j)): p for p, (i, j) in enumerate(zip(II, JJ))}
    rom = np.zeros(HP, dtype=np.int64)
    for mh, (i, j) in enumerate(pairs):
        rom[mh] = p2r[(i, j)]
        rom[105 + mh] = 105 + p2r[(i, j)]
    for i in range(15):
        rom[210 + i] = 210 + i
        rom[225 + i] = 225 + i
        rom[240 + i] = 240 + i
        rom[255 + i] = 255 + i
    return rom


def _row_alpha_beta():
    alpha = np.zeros(HP)
    beta = np.zeros(HP)
    alpha[0:105] = 1.0 / 0.9
    alpha[105:210] = 9.0
    alpha[210:225] = 3.0
    alpha[225:240] = 10.0 / 3.0
    alpha[240:255] = 1.0
    beta[240:255] = 1.0
    alpha[255:270] = 1.0
    return alpha, beta


def _conv_pieces():
    plan = []
    for t in range(NT):
        r0, r1 = 128 * t, 128 * t + 127
        h0 = r0 // 160
        hl = min(r1 // 160, HP - 1)
        need = list(range(h0, hl + 1))
        if h0 % 2 == 0:
            pieces = [(h0, need)]
        elif len(need) == 1:
            pieces = [(h0 - 1, [h0])]
        else:
            pieces = [(h0 - 1, [h0]), (h0 + 1, [h0 + 1])]
        plan.append(pieces)
    return plan


def _build_device_inputs(inp):
    gamma = float(inp['bn_gamma'][0]); betab = float(inp['bn_beta'][0])
    mu = float(inp['bn_mean'][0]); var = float(inp['bn_var'][0])
    a = gamma / np.sqrt(var + BN_EPS)
    b = betab - mu * a
    conv_w = np.asarray(inp['conv_w'], np.float64).reshape(16, 3)
    conv_b = np.asarray(inp['conv_b'], np.float64)

    alpha, beta = _row_alpha_beta()
    sA = a / alpha
    sB = b - a * beta / alpha

    ybias = np.zeros(K1PAD, np.float64)
    wsum = conv_w.sum(axis=1)
    for mh in range(HP):
        ybias[mh * 160:(mh + 1) * 160] = np.repeat(conv_b + wsum * sB[mh], 10)
    ybias2d = ybias.reshape(NT, 128).T.astype(np.float32).copy()

    rom = _ref_perm()
    m = np.arange(K1)
    mh = m // 160; o = (m % 160) // 10; wp = m % 10
    kref = o * 2700 + rom[mh] * 10 + wp
    fc1_w = np.asarray(inp['fc1_w'], np.float32)
    fc1t = np.zeros((K1PAD, 512), np.float32)
    fc1t[:K1, :] = fc1_w[:, kref].T
    fc1t = fc1t.astype(bf16)

    # One full-height (K=128) matmul per conv tile against its f-tile u;
    # A streamed from DRAM per tile.
    piece_refs = []
    A_packed = np.zeros((NT, 128, 128), np.float32)
    for t in range(NT):
        r0, r1 = 128 * t, 128 * t + 127
        h0 = r0 // 160
        hl = min(r1 // 160, HP - 1)
        hs = list(range(h0, hl + 1))
        u = h0 // 8                       # f tile holds h in [8u, 8u+8)
        assert all(8 * u <= h < 8 * u + 8 for h in hs), (t, hs, u)
        for col in range(128):
            mm = 128 * t + col
            if mm >= K1:
                continue
            mhh = mm // 160
            if mhh not in hs:
                continue
            oo = (mm % 160) // 10
            wpp = mm % 10
            for k in range(3):
                A_packed[t, (mhh - 8 * u) * 16 + wpp + k, col] = \
                    conv_w[oo, k] * sA[mhh]
        piece_refs.append(u)
    A_packed = A_packed.astype(bf16)
    ncol = 0

    fc1b2d = np.asarray(inp['fc1_b'], np.float32).reshape(4, 128).T.copy()
    fc2t = np.ascontiguousarray(
        np.asarray(inp['fc2_w'], np.float32).T).astype(bf16)
    fc2b = np.asarray(inp['fc2_b'], np.float32).reshape(128, 1).copy()
    fc3t = np.ascontiguousarray(
        np.asarray(inp['fc3_w'], np.float32).reshape(1, 128).T).astype(bf16)
    fc3b = np.asarray(inp['fc3_b'], np.float32).reshape(1, 1).copy()
    wdl = np.tile((np.arange(1, 11, dtype=np.float32) / 55.0)[None, :],
                  (128, 1)).astype(bf16)

    return dict(A_packed=A_packed, ncol=ncol, piece_refs=piece_refs,
                ybias2d=ybias2d, fc1t=fc1t, fc1b2d=fc1b2d,
                fc2t=fc2t, fc2b=fc2b, fc3t=fc3t, fc3b=fc3b, wdl=wdl)


# ------------------------- device kernel builder -------------------------

def build_nc(ncol, piece_refs, stage=3):
    nc = bacc.Bacc("TRN2", target_bir_lowering=False, debug=False,
                   num_devices=NCORES)
    f32, b16 = dt.float32, dt.bfloat16
    data_e = nc.declare_dram_parameter("data", [NSH, F * W], b16, isOutput=False)
    A_e = nc.declare_dram_parameter("A_packed", [NT, 128, 128], b16, isOutput=False)
    fc1t_e = nc.declare_dram_parameter("fc1t", [K1PAD, 512], b16, isOutput=False)
    yb_e = nc.declare_dram_parameter("ybias2d", [128, NT], f32, isOutput=False)
    fc1b_e = nc.declare_dram_parameter("fc1b2d", [128, 4], f32, isOutput=False)
    fc2t_e = nc.declare_dram_parameter("fc2t", [512, 128], b16, isOutput=False)
    fc2b_e = nc.declare_dram_parameter("fc2b", [128, 1], f32, isOutput=False)
    fc3t_e = nc.declare_dram_parameter("fc3t", [128, 1], b16, isOutput=False)
    fc3b_e = nc.declare_dram_parameter("fc3b", [1, 1], f32, isOutput=False)
    wdl_e = nc.declare_dram_parameter("wdl", [128, 10], b16, isOutput=False)
    out_e = nc.declare_dram_parameter("out", [1, NSH], f32, isOutput=True)

    _, cb = _mine_row_tables()
    AF = mybir.ActivationFunctionType

    with tile.TileContext(nc) as tc, ExitStack() as ctx:
        consts = ctx.enter_context(tc.tile_pool(name="consts", bufs=1))
        fpool = ctx.enter_context(tc.tile_pool(name="fpool", bufs=1))
        datap = ctx.enter_context(tc.tile_pool(name="datap", bufs=4))
        featp = ctx.enter_context(tc.tile_pool(name="featp", bufs=2))
        spreadp = ctx.enter_context(tc.tile_pool(name="spreadp", bufs=2))
        meansump = ctx.enter_context(tc.tile_pool(name="meansump", bufs=2))
        rstdp = ctx.enter_context(tc.tile_pool(name="rstdp", bufs=2))
        scratch = ctx.enter_context(tc.tile_pool(name="scratch", bufs=4))
        xpool = ctx.enter_context(tc.tile_pool(name="xpool", bufs=6))
        w1pool = ctx.enter_context(tc.tile_pool(name="w1pool", bufs=8))
        apool = ctx.enter_context(tc.tile_pool(name="apool", bufs=8))
        x2pool = ctx.enter_context(tc.tile_pool(name="x2pool", bufs=1))
        outp = ctx.enter_context(tc.tile_pool(name="outp", bufs=1))
        ps_fc1 = ctx.enter_context(tc.tile_pool(name="ps_fc1", bufs=1, space="PSUM"))
        ps_conv = ctx.enter_context(tc.tile_pool(name="ps_conv", bufs=4, space="PSUM"))

        # data DMAs first so stats start immediately
        dtiles = []
        for bkl in range(NB):
            d = datap.tile([128, F, NW, S], b16, tag="d", name=f"d{bkl}")
            nc.sync.dma_start(
                d[:], data_e[128 * bkl:128 * (bkl + 1), :]
                .rearrange("p (f nw s) -> p f nw s", f=F, nw=NW))
            dtiles.append(d)

        # constants
        yb_sb = consts.tile([128, NT], f32)
        nc.sync.dma_start(yb_sb[:], yb_e[:])
        fc1b_sb = consts.tile([128, 4], f32)
        nc.sync.dma_start(fc1b_sb[:], fc1b_e[:])
        fc2t_sb = consts.tile([128, 4, 128], b16)
        nc.sync.dma_start(fc2t_sb[:], fc2t_e.rearrange("(kb k) j -> k kb j", k=128))
        fc2b_sb = consts.tile([128, 1], f32)
        nc.sync.dma_start(fc2b_sb[:], fc2b_e[:])
        fc3t_sb = consts.tile([128, 1], b16)
        nc.sync.dma_start(fc3t_sb[:], fc3t_e[:])
        fc3b_sb = consts.tile([1, 1], f32)
        nc.sync.dma_start(fc3b_sb[:], fc3b_e[:])
        wdl_sb = consts.tile([128, 10], b16)
        nc.sync.dma_start(wdl_sb[:], wdl_e[:])
        ident = consts.tile([128, 128], f32)
        make_identity(nc, ident[:])

        # persistent bf16 transposed-feature buffer [row, sample]
        f_sb = fpool.tile([128, NGT, NSH], b16)

        # ---------------- per-block stats + transpose ----------------
        for bkl in range(NB):
            d = dtiles[bkl]
            feat = featp.tile([128, NROW_PAD, WPAD], f32)
            # zero only the padding (w cols 12:16 and rows 270:272)
            nc.vector.memset(feat[:, :, NW:WPAD], 0.0)
            nc.vector.memset(feat[:, HP:NROW_PAD, 0:NW], 0.0)

            meansum = meansump.tile([128, F, NW], f32)
            nc.vector.tensor_reduce(meansum[:], d[:], axis=mybir.AxisListType.X,
                                    op=mybir.AluOpType.add)
            mean = scratch.tile([128, F, NW], b16, tag="s180")
            nc.vector.tensor_scalar_mul(mean[:], meansum[:], 1.0 / S)
            spread = spreadp.tile([128, F, NW, S], b16)
            nc.vector.tensor_sub(
                spread[:], d[:], mean[:, :, :, None].to_broadcast((128, F, NW, S)))

            varsum = scratch.tile([128, F, NW], f32, tag="s180f")
            prod = scratch.tile([128, F, NW, S], b16, tag="prod")
            nc.vector.tensor_mul(prod[:], spread[:], spread[:])
            nc.vector.tensor_reduce(varsum[:], prod[:], axis=mybir.AxisListType.X,
                                    op=mybir.AluOpType.add)
            nc.scalar.activation(feat[:, 210:225, 0:NW], varsum[:], AF.Sqrt,
                                 bias=0.0, scale=1.0)
            rstd = rstdp.tile([128, F, NW], f32)
            nc.vector.reciprocal(rstd[:], feat[:, 210:225, 0:NW])
            # zscore
            nc.vector.tensor_mul(feat[:, 225:240, 0:NW], meansum[:], rstd[:])
            # return: last/first
            recipf = scratch.tile([128, F, NW], f32, tag="s180f")
            nc.vector.reciprocal(recipf[:], d[:, :, :, 0])
            nc.vector.tensor_mul(feat[:, 240:255, 0:NW], d[:, :, :, S - 1], recipf[:])
            # decay-linear
            dlp = scratch.tile([128, F, NW, S], b16, tag="prod")
            nc.vector.tensor_mul(
                dlp[:], d[:], wdl_sb[:, None, None, :].to_broadcast((128, F, NW, S)))
            nc.vector.tensor_reduce(feat[:, 255:270, 0:NW], dlp[:],
                                    axis=mybir.AxisListType.X, op=mybir.AluOpType.add)
            # cov + corr per offset d
            for dd in range(1, 15):
                nf = 15 - dd
                cp = scratch.tile([128, nf, NW, S], b16, tag="prod")
                nc.vector.tensor_mul(cp[:], spread[:, 0:nf], spread[:, dd:15])
                cov_slice = feat[:, 105 + cb[dd]:105 + cb[dd] + nf, 0:NW]
                nc.vector.tensor_reduce(cov_slice, cp[:],
                                        axis=mybir.AxisListType.X,
                                        op=mybir.AluOpType.add)
                rsp = scratch.tile([128, nf, NW], f32, tag="s180f")
                nc.vector.tensor_mul(rsp[:], rstd[:, 0:nf], rstd[:, dd:15])
                nc.vector.tensor_mul(feat[:, cb[dd]:cb[dd] + nf, 0:NW],
                                     cov_slice, rsp[:])

            # transpose this block's features into f_sb
            featf = feat.rearrange("p r w -> p (r w)")
            for u in range(NGT):
                tp = ps_conv.tile([128, NSH], f32, tag="cps",
                                  name=f"tp{bkl}_{u}")
                nc.tensor.transpose(tp[:, 0:128],
                                    featf[:, 128 * u:128 * (u + 1)], ident[:])
                nc.scalar.activation(f_sb[:, u, 128 * bkl:128 * (bkl + 1)],
                                     tp[:, 0:128], AF.Copy, bias=0.0, scale=1.0)

        if stage <= 1:
            out_sb1 = outp.tile([1, NSH], f32)
            nc.vector.tensor_copy(out_sb1[:], f_sb[0:1, 0, :])
            nc.sync.dma_start(out_e[:], out_sb1[:])

        # ---------------- conv + fc1 stream ----------------
        fc1ps = [ps_fc1.tile([128, NSH], f32, tag=f"jb{jb}", name=f"fc1ps{jb}")
                 for jb in range(4)] if stage >= 3 else None
        last_xt = None
        for t in range(NT if stage >= 2 else 0):
            cps = ps_conv.tile([128, NSH], f32, tag="cps", name=f"cps{t}")
            u = piece_refs[t]
            a1 = apool.tile([128, 128], b16)
            nc.sync.dma_start(a1[:], A_e[t])
            nc.tensor.matmul(cps[:], a1[:], f_sb[:, u, :],
                             start=True, stop=True)
            xt = xpool.tile([128, NSH], b16)
            nc.scalar.activation(xt[:], cps[:], AF.Relu,
                                 bias=yb_sb[:, t:t + 1], scale=1.0)
            last_xt = xt
            if stage >= 3:
                w1 = w1pool.tile([128, 512], b16)
                nc.sync.dma_start(w1[:], fc1t_e[128 * t:128 * (t + 1), :])
                for jb in range(4):
                    nc.tensor.matmul(fc1ps[jb][:],
                                     w1[:, 128 * jb:128 * (jb + 1)],
                                     xt[:], start=(t == 0), stop=(t == NT - 1))

        if stage == 2:
            out_sb2 = outp.tile([1, NSH], f32)
            nc.vector.tensor_copy(out_sb2[:], last_xt[0:1, :])
            nc.sync.dma_start(out_e[:], out_sb2[:])

        # ---------------- fc2 / fc3 ----------------
        x2 = (x2pool.tile([128, 4, NSH], b16, name="x2")
              if stage >= 3 else None)
        for jb in range(4 if stage >= 3 else 0):
            nc.scalar.activation(x2[:, jb, :], fc1ps[jb][:], AF.Relu,
                                 bias=fc1b_sb[:, jb:jb + 1], scale=1.0)
        if stage >= 3:
            fc2ps = ps_fc1.tile([128, NSH], f32, tag="jb0")
            for kb in range(4):
                nc.tensor.matmul(fc2ps[:], fc2t_sb[:, kb, :], x2[:, kb, :],
                                 start=(kb == 0), stop=(kb == 3))
            x3 = x2pool.tile([128, NSH], b16)
            nc.scalar.activation(x3[:], fc2ps[:], AF.Sigmoid,
                                 bias=fc2b_sb[:], scale=1.0)
            fc3ps = ps_fc1.tile([128, NSH], f32, tag="jb1")
            nc.tensor.matmul(fc3ps[0:1, :], fc3t_sb[:], x3[:],
                             start=True, stop=True)
            out_sb = outp.tile([1, NSH], f32)
            nc.scalar.activation(out_sb[:], fc3ps[0:1, :], AF.Identity,
                                 bias=fc3b_sb[:], scale=1.0)
            nc.sync.dma_start(out_e[:], out_sb[:])

    nc.compile()
    return nc


# ------------------------------- entry -------------------------------

def _prep_in_maps(inputs):
    dev = _build_device_inputs(inputs)
    data = np.ascontiguousarray(
        np.asarray(inputs['data'], np.float32).reshape(NFULL, F * W)).astype(bf16)
    shared = {k: dev[k] for k in ('A_packed', 'ybias2d', 'fc1t', 'fc1b2d',
                                  'fc2t', 'fc2b', 'fc3t', 'fc3b', 'wdl')}
    in_maps = []
    for c in range(NCORES):
        m = dict(shared)
        m['data'] = data[NSH * c:NSH * (c + 1)]
        in_maps.append(m)
    return dev, in_maps


def run(inputs, trace=False, tmpdir=None):
    dev, in_maps = _prep_in_maps(inputs)
    nc = build_nc(dev['ncol'], dev['piece_refs'])
    res = run_bass_kernel_spmd(nc, in_maps, core_ids=list(range(NCORES)),
                               trace=trace, tmpdir=tmpdir)
    out = np.concatenate([np.asarray(r["out"], np.float32).reshape(NSH)
                          for r in res.results])
    return out, res


def kernel(**inputs) -> np.ndarray:
    out, _ = run(inputs, trace=False)
    return out
